# revision 14
# baseline (speedup 1.0000x reference)
"""Trainium2 Bass kernel for nn_LrUpsampling (TransformerConv + GraphNorm + cosine gram).

Sharding: node-parallel over 8 cores, two collectives total.
- Each core owns a 512-node slice of the N=4096 query axis and computes
  attention for all 4 heads over its queries (K/V computed redundantly
  over all source nodes from the full lr_x).
- GraphNorm moments: per-core partial sums over own nodes, one tiny
  AllReduce ([2, 2048] f32).
- GraphNorm scale/bias and the cosine row/col normalization fold into a
  single per-channel affine y = s*h + b (diag of the gram is derived
  analytically from the moments, no extra data pass).
- Gram: each core computes the full [2048, 2048] partial gram over its
  own 512 nodes; one ReduceScatter sums partials and hands each core its
  own 256 output rows. relu, done.

All matmuls run as float32r (PE full-rate fp32-reduced).
"""
import numpy as np

LR, HR, HEADS = 512, 2048, 4
C = HR // HEADS          # 512 per-head channels
N = 2 * HR               # 4096 nodes
NO = N // 8              # 512 own nodes per core
EPS = 1e-5
N_CORES = 8
SCALE = 1.0 / np.sqrt(np.float32(C))

_RUNNER = None


def _build(stop_after=None):
    import os
    stop_after = stop_after or os.environ.get("K_STOP_AFTER") or None
    from concourse import bacc, tile, mybir
    from concourse.masks import make_identity

    f32 = mybir.dt.float32
    f32r = mybir.dt.float32r
    bf16 = mybir.dt.bfloat16
    AF = mybir.ActivationFunctionType
    ALU = mybir.AluOpType
    ALL = [list(range(N_CORES))]

    nc = bacc.Bacc("TRN2", target_bir_lowering=False, debug=False,
                   num_devices=N_CORES)

    # ---- I/O ----
    x = nc.dram_tensor("x", [LR, N], f32r, kind="ExternalInput")     # full lr_x
    xo = nc.dram_tensor("xo", [LR, NO], f32r, kind="ExternalInput")  # own cols
    wq = nc.dram_tensor("wq", [LR, HR], f32r, kind="ExternalInput")
    wk = nc.dram_tensor("wk", [LR, HR], f32r, kind="ExternalInput")
    wv = nc.dram_tensor("wv", [LR, HR], f32r, kind="ExternalInput")
    ws = nc.dram_tensor("ws", [LR, HR], f32r, kind="ExternalInput")
    # per-channel columns [p, kind, blk]: 0=bq 1=bk 2=bv+bskip  (ch = blk*128+p)
    cols = nc.dram_tensor("cols", [128, 3, 16], f32, kind="ExternalInput")
    # per-channel rows: 0=gn_weight 1=gn_bias 2=gn_mean_scale
    rows = nc.dram_tensor("rows", [3, HR], f32, kind="ExternalInput")
    g_out = nc.dram_tensor("g", [256, HR], f32, kind="ExternalOutput")

    with tile.TileContext(nc) as tc:
        import contextlib
        ctx = contextlib.ExitStack()
        with ctx:
            consts = ctx.enter_context(tc.tile_pool(name="consts", bufs=1))
            dram = ctx.enter_context(tc.tile_pool(name="dram", bufs=1, space="DRAM"))

            # ---- constants ----
            ident = consts.tile([128, 128], f32)
            make_identity(nc, ident[:])
            ones_f = consts.tile([128, 1], f32)
            nc.vector.memset(ones_f[:], 1.0)
            ones_col = consts.tile([128, 1], f32r)
            nc.scalar.copy(ones_col[:], ones_f[:])
            onesr_f = consts.tile([1, 128], f32)
            nc.vector.memset(onesr_f[:], 1.0)
            ones_row = consts.tile([1, 128], f32r)
            nc.scalar.copy(ones_row[:], onesr_f[:])
            eps_c = consts.tile([1, 1], f32)
            nc.vector.memset(eps_c[:], EPS)
            cols_sb = consts.tile([128, 3, 16], f32)
            nc.sync.dma_start(cols_sb[:], cols.ap())

            v_dram = [dram.tile([N, C], f32r, name=f"vd{h}") for h in range(4)]

            # pool for tiles that outlive the per-head phases (opened first
            # so later pools close in stack order)
            hs = ctx.enter_context(tc.tile_pool(name="hs", bufs=1))

            # ============ Phase 1+2 per head: projections + attention ======
            hp_cm = tc.tile_pool(name="hp", bufs=1)
            hp = hp_cm.__enter__()
            h_all = hp.tile([128, 16, NO], f32)     # [ch-part, h*4+cc, own n] 4MB

            pa_cm = tc.tile_pool(name="pa", bufs=1)
            pa = pa_cm.__enter__()
            for h in range(4):
                kT = pa.tile([128, 4, N], f32r, tag="kt", name=f"kt{h}")
                qT = pa.tile([128, 4, NO], f32r, tag="qt", name=f"qt{h}")
                skT = pa.tile([128, 4, NO], f32, tag="sk", name=f"sk{h}")
                with tc.tile_pool(name=f"p1s{h}", bufs=2) as p1s, \
                     tc.tile_pool(name=f"p1w{h}", bufs=1) as p1w, \
                     tc.tile_pool(name=f"p1p{h}", bufs=4, space="PSUM") as p1p:
                    wk_sb = p1w.tile([128, 4, C], f32r, tag="w1", name=f"wk{h}")
                    wv_sb = p1w.tile([128, 4, C], f32r, tag="w2", name=f"wv{h}")
                    nc.sync.dma_start(
                        wk_sb[:], wk.ap().rearrange("(l p) c -> p l c", p=128)
                        [:, :, h * C:(h + 1) * C])
                    nc.sync.dma_start(
                        wv_sb[:], wv.ap().rearrange("(l p) c -> p l c", p=128)
                        [:, :, h * C:(h + 1) * C])
                    # kT (all nodes) and v (all nodes) from one x pass
                    for mm8 in range(8):
                        x_t = p1s.tile([128, 4, 512], f32r, tag="xs",
                                       name=f"x{h}_{mm8}")
                        nc.sync.dma_start(
                            x_t[:], x.ap().rearrange("(l p) m -> p l m", p=128)
                            [:, :, mm8 * 512:(mm8 + 1) * 512])
                        for cc in range(4):
                            ps = p1p.tile([128, 512], f32, tag="ps")
                            for lc in range(4):
                                nc.tensor.matmul(
                                    ps[:], wk_sb[:, lc, cc * 128:(cc + 1) * 128],
                                    x_t[:, lc, :], start=(lc == 0), stop=(lc == 3))
                            nc.vector.tensor_scalar_add(
                                kT[:, cc, mm8 * 512:(mm8 + 1) * 512], ps[:],
                                cols_sb[:, 1, h * 4 + cc:h * 4 + cc + 1])
                        for sub in range(4):
                            ps = p1p.tile([128, 512], f32, tag="ps")
                            for lc in range(4):
                                nc.tensor.matmul(
                                    ps[:], x_t[:, lc, sub * 128:(sub + 1) * 128],
                                    wv_sb[:, lc, :], start=(lc == 0), stop=(lc == 3))
                            v_st = p1s.tile([128, 512], f32r, tag="vst")
                            nc.vector.tensor_copy(v_st[:], ps[:])
                            nc.sync.dma_start(
                                v_dram[h][mm8 * 512 + sub * 128:
                                          mm8 * 512 + (sub + 1) * 128, :], v_st[:])
                    # qT and skipT over own nodes (reuse weight slots)
                    wq_sb = p1w.tile([128, 4, C], f32r, tag="w1", name=f"wq{h}")
                    ws_sb = p1w.tile([128, 4, C], f32r, tag="w2", name=f"ws{h}")
                    nc.sync.dma_start(
                        wq_sb[:], wq.ap().rearrange("(l p) c -> p l c", p=128)
                        [:, :, h * C:(h + 1) * C])
                    nc.sync.dma_start(
                        ws_sb[:], ws.ap().rearrange("(l p) c -> p l c", p=128)
                        [:, :, h * C:(h + 1) * C])
                    xo_t = p1s.tile([128, 4, NO], f32r, tag="xs", name=f"xo{h}")
                    nc.sync.dma_start(
                        xo_t[:], xo.ap().rearrange("(l p) m -> p l m", p=128))
                    for cc in range(4):
                        ps = p1p.tile([128, 512], f32, tag="ps")
                        for lc in range(4):
                            nc.tensor.matmul(
                                ps[:], wq_sb[:, lc, cc * 128:(cc + 1) * 128],
                                xo_t[:, lc, :], start=(lc == 0), stop=(lc == 3))
                        nc.vector.tensor_scalar_add(
                            qT[:, cc, :], ps[:],
                            cols_sb[:, 0, h * 4 + cc:h * 4 + cc + 1])
                        ps2 = p1p.tile([128, 512], f32, tag="ps")
                        for lc in range(4):
                            nc.tensor.matmul(
                                ps2[:], ws_sb[:, lc, cc * 128:(cc + 1) * 128],
                                xo_t[:, lc, :], start=(lc == 0), stop=(lc == 3))
                        nc.vector.tensor_scalar_add(
                            skT[:, cc, :], ps2[:],
                            cols_sb[:, 2, h * 4 + cc:h * 4 + cc + 1])

                # -------- attention for head h, own 512 queries --------
                with tc.tile_pool(name=f"p2s{h}", bufs=2) as p2s, \
                     tc.tile_pool(name=f"p2b{h}", bufs=1) as p2b, \
                     tc.tile_pool(name=f"p2ps{h}", bufs=2, space="PSUM") as p2ps, \
                     tc.tile_pool(name=f"p2po{h}", bufs=1, space="PSUM") as p2po:
                    o_ps = [p2po.tile([128, 512], f32, tag=f"o{cc}",
                                      name=f"o{h}_{cc}")
                            for cc in range(4)]
                    den_ps = p2po.tile([1, 512], f32, tag="den")
                    for mb in range(32):
                        if mb % 8 == 0:
                            v8 = p2s.tile([128, 8, 512], f32r, tag="vt",
                                          name=f"v8_{h}_{mb}")
                            nc.sync.dma_start(
                                v8[:], v_dram[h][mb * 128:(mb + 8) * 128, :]
                                .rearrange("(i p) c -> p i c", p=128))
                        s_ps = p2ps.tile([128, 512], f32, tag="s")
                        for cc in range(4):
                            nc.tensor.matmul(
                                s_ps[:], kT[:, cc, mb * 128:(mb + 1) * 128],
                                qT[:, cc, :], start=(cc == 0), stop=(cc == 3))
                        e_t = p2s.tile([128, 512], f32r, tag="e")
                        nc.scalar.activation(e_t[:], s_ps[:], AF.Exp,
                                             scale=float(SCALE))
                        for cc in range(4):
                            nc.tensor.matmul(
                                o_ps[cc][:],
                                v8[:, mb % 8, cc * 128:(cc + 1) * 128], e_t[:],
                                start=(mb == 0), stop=(mb == 31))
                        nc.tensor.matmul(den_ps[:], ones_col[:], e_t[:],
                                         start=(mb == 0), stop=(mb == 31))
                    rec_f = p2b.tile([1, 512], f32, tag="rec")
                    nc.vector.reciprocal(rec_f[:], den_ps[:])
                    rec_r = p2b.tile([1, 512], f32r, tag="recr")
                    nc.scalar.copy(rec_r[:], rec_f[:])
                    bc_ps = p2po.tile([128, 512], f32, tag="bc")
                    nc.tensor.matmul(bc_ps[:], ones_row[:], rec_r[:],
                                     start=True, stop=True)
                    bc_sb = p2b.tile([128, 512], f32, tag="bcs")
                    nc.vector.tensor_copy(bc_sb[:], bc_ps[:])
                    for cc in range(4):
                        nc.vector.tensor_tensor(
                            h_all[:, h * 4 + cc, :], o_ps[cc][:], bc_sb[:],
                            op=ALU.mult)
                        nc.vector.tensor_tensor(
                            h_all[:, h * 4 + cc, :], h_all[:, h * 4 + cc, :],
                            skT[:, cc, :], op=ALU.add)
            pa_cm.__exit__(None, None, None)

            # ============ Phase 3: transpose to node-major ============
            y_sb = hs.tile([128, 4, HR], f32r)     # [n-part, nn, ch] 4MB
            with tc.tile_pool(name="tp", bufs=4, space="PSUM") as tpp:
                for hc in range(16):
                    for nn in range(4):
                        tp = tpp.tile([128, 128], f32, tag="tp")
                        nc.tensor.transpose(
                            tp[:], h_all[:, hc, nn * 128:(nn + 1) * 128], ident[:])
                        nc.vector.tensor_copy(
                            y_sb[:, nn, hc * 128:(hc + 1) * 128], tp[:])
            hp_cm.__exit__(None, None, None)

            if stop_after == "h":
                hdump = hs.tile([128, 2, HR], f32, name="hdump")
                nc.vector.tensor_copy(hdump[:], y_sb[:, 0:2, :])
                nc.sync.dma_start(
                    g_out.ap().rearrange("(r p) k -> p r k", p=128), hdump[:])

            if stop_after != "h":
                # ============ Phase 4: moments + AllReduce ============
                rws = ctx.enter_context(tc.tile_pool(name="rws", bufs=1))
                # engine ops must start at partition 0 -> one [1, HR] tile per row
                gam_sb = rws.tile([1, HR], f32)
                nc.sync.dma_start(gam_sb[:], rows.ap()[0:1, :])
                bet_sb = rws.tile([1, HR], f32)
                nc.sync.dma_start(bet_sb[:], rows.ap()[1:2, :])
                ms_sb = rws.tile([1, HR], f32)
                nc.sync.dma_start(ms_sb[:], rows.ap()[2:3, :])
                with tc.tile_pool(name="mp", bufs=1, space="PSUM") as mp, \
                     tc.tile_pool(name="msx", bufs=2) as msp:
                    mom_ps = mp.tile([1, HR], f32, tag="mom")
                    sq_ps = mp.tile([1, HR], f32, tag="sq")
                    for nn in range(4):
                        hsq = msp.tile([128, HR], f32r, tag="hsq")
                        nc.scalar.square(hsq[:], y_sb[:, nn, :])
                        for s4 in range(4):
                            nc.tensor.matmul(
                                mom_ps[:, s4 * 512:(s4 + 1) * 512], ones_col[:],
                                y_sb[:, nn, s4 * 512:(s4 + 1) * 512],
                                start=(nn == 0), stop=(nn == 3))
                            nc.tensor.matmul(
                                sq_ps[:, s4 * 512:(s4 + 1) * 512], ones_col[:],
                                hsq[:, s4 * 512:(s4 + 1) * 512],
                                start=(nn == 0), stop=(nn == 3))
                    mom_sb = rws.tile([1, HR], f32, name="mom_sb")
                    sq_sb = rws.tile([1, HR], f32, name="sq_sb")
                    nc.vector.tensor_copy(mom_sb[:], mom_ps[:])
                    nc.vector.tensor_copy(sq_sb[:], sq_ps[:])
                    mom_in = dram.tile([2, HR], f32)
                    mom_out = dram.tile([2, HR], f32)
                    nc.sync.dma_start(mom_in[0:1, :], mom_sb[:])
                    nc.sync.dma_start(mom_in[1:2, :], sq_sb[:])
                if stop_after == "momnc":
                    nc.sync.dma_start(mom_out[:], mom_in[:])
                else:
                    nc.gpsimd.collective_compute(
                        "AllReduce", ALU.add, replica_groups=ALL,
                        ins=[mom_in.opt()], outs=[mom_out.opt()])
                mom_g = rws.tile([1, HR], f32)
                nc.sync.dma_start(mom_g[:], mom_out[0:1, :])
                sq_g = rws.tile([1, HR], f32)
                nc.sync.dma_start(sq_g[:], mom_out[1:2, :])

                # ---- fused affine: y = sA*h + bA (6 scratch rows r0..r5) ----
                mom_r = mom_g[:]
                sq_r = sq_g[:]
                r0 = rws.tile([1, HR], f32, name="r0")
                r1 = rws.tile([1, HR], f32, name="r1")
                r2 = rws.tile([1, HR], f32, name="r2")
                r3 = rws.tile([1, HR], f32, name="r3")
                r4 = rws.tile([1, HR], f32, name="r4")
                r5 = rws.tile([1, HR], f32, name="r5")
                TT = nc.vector.tensor_tensor
                nc.scalar.mul(r0[:], mom_r, 1.0 / N)                    # mean
                nc.scalar.mul(r1[:], sq_r, 1.0 / N)                     # ex2
                TT(r2[:], ms_sb[:], r0[:], op=ALU.mult)          # t = ms*mean
                nc.scalar.mul(r3[:], r0[:], 2.0)
                TT(r3[:], r3[:], r2[:], op=ALU.subtract)                # u = 2m - t
                TT(r3[:], r2[:], r3[:], op=ALU.mult)                    # t*u
                TT(r1[:], r1[:], r3[:], op=ALU.subtract)                # var
                nc.scalar.activation(r3[:], r1[:], AF.Sqrt, bias=eps_c[:])
                nc.vector.reciprocal(r1[:], r3[:])                      # rstd
                TT(r4[:], gam_sb[:], r1[:], op=ALU.mult)          # sY
                TT(r5[:], r2[:], r4[:], op=ALU.mult)
                TT(r5[:], bet_sb[:], r5[:], op=ALU.subtract)      # bY
                # diag = sY^2*sq + 2*sY*bY*mom + N*bY^2
                TT(r2[:], r4[:], r4[:], op=ALU.mult)
                TT(r2[:], r2[:], sq_r, op=ALU.mult)                     # d1
                TT(r3[:], r4[:], r5[:], op=ALU.mult)
                TT(r3[:], r3[:], mom_r, op=ALU.mult)
                nc.scalar.mul(r3[:], r3[:], 2.0)                        # d2
                TT(r2[:], r2[:], r3[:], op=ALU.add)
                TT(r3[:], r5[:], r5[:], op=ALU.mult)
                nc.scalar.mul(r3[:], r3[:], float(N))                   # d3
                TT(r2[:], r2[:], r3[:], op=ALU.add)                     # diag
                nc.scalar.activation(r3[:], r2[:], AF.Sqrt)
                nc.vector.reciprocal(r2[:], r3[:])                      # rA
                TT(r4[:], r4[:], r2[:], op=ALU.mult)                    # sA
                TT(r5[:], r5[:], r2[:], op=ALU.mult)                    # bA
                sA_r = rws.tile([1, HR], f32r)
                nc.scalar.copy(sA_r[:], r4[:])
                bA_r = rws.tile([1, HR], f32r)
                nc.scalar.copy(bA_r[:], r5[:])

                # broadcast rows to [128, HR]
                bcs = rws.tile([128, HR], f32r, tag="big1", name="bcs")
                bcb = rws.tile([128, HR], f32r, tag="big2", name="bcb")
                with tc.tile_pool(name="bcp", bufs=1, space="PSUM") as bcp:
                    bs_ps = bcp.tile([128, HR], f32, tag="bs")
                    bb_ps = bcp.tile([128, HR], f32, tag="bb")
                    for s4 in range(4):
                        nc.tensor.matmul(
                            bs_ps[:, s4 * 512:(s4 + 1) * 512], ones_row[:],
                            sA_r[:, s4 * 512:(s4 + 1) * 512], start=True, stop=True)
                        nc.tensor.matmul(
                            bb_ps[:, s4 * 512:(s4 + 1) * 512], ones_row[:],
                            bA_r[:, s4 * 512:(s4 + 1) * 512], start=True, stop=True)
                    nc.vector.tensor_copy(bcs[:], bs_ps[:])
                    nc.vector.tensor_copy(bcb[:], bb_ps[:])

                # normalize own rows in place
                for nn in range(4):
                    nc.vector.tensor_tensor(y_sb[:, nn, :], y_sb[:, nn, :],
                                            bcs[:], op=ALU.mult)
                    nc.vector.tensor_tensor(y_sb[:, nn, :], y_sb[:, nn, :],
                                            bcb[:], op=ALU.add)

                if stop_after in ("mom", "momnc", "norm"):
                    if stop_after == "norm":
                        nc.sync.dma_start(
                            g_out.ap().rearrange("(r p) k -> p r k", p=128),
                            y_sb[:, 0:2, :])
                    else:
                        dummy = rws.tile([128, 2, HR], f32, name="dummy")
                        nc.vector.memset(dummy[:], 0.0)
                        nc.vector.tensor_copy(dummy[0:1, 0, :], mom_g[:])
                        nc.sync.dma_start(
                            g_out.ap().rearrange("(r p) k -> p r k", p=128),
                            dummy[:])
                else:
                    # ============ Phase 5: partial gram + ReduceScatter ====
                    zpart = dram.tile([HR, HR], bf16)
                    zred = dram.tile([256, HR], bf16)
                    with tc.tile_pool(name="zp", bufs=2, space="PSUM") as zp, \
                         tc.tile_pool(name="zs", bufs=2) as zs:
                        for rb in range(16):
                            z_ps = zp.tile([128, HR], f32, tag="z")
                            for nn in range(4):
                                for s4 in range(4):
                                    nc.tensor.matmul(
                                        z_ps[:, s4 * 512:(s4 + 1) * 512],
                                        y_sb[:, nn, rb * 128:(rb + 1) * 128],
                                        y_sb[:, nn, s4 * 512:(s4 + 1) * 512],
                                        start=(nn == 0), stop=(nn == 3))
                            zrow = zs.tile([128, HR], bf16, tag="zr")
                            nc.vector.tensor_copy(zrow[:], z_ps[:])
                            nc.sync.dma_start(
                                zpart[rb * 128:(rb + 1) * 128, :], zrow[:])
                    if stop_after == "zpart":
                        zfirst = rws.tile([128, 2, HR], f32, name="zfirst")
                        nc.sync.dma_start(
                            zfirst[:],
                            zpart[0:256, :].rearrange("(r p) k -> p r k", p=128))
                        nc.sync.dma_start(
                            g_out.ap().rearrange("(r p) k -> p r k", p=128),
                            zfirst[:])
                    else:
                        nc.gpsimd.collective_compute(
                            "ReduceScatter", ALU.add, replica_groups=ALL,
                            ins=[zpart.opt()], outs=[zred.opt()])
                        gb = rws.tile([128, 2, HR], bf16, tag="big1", name="gb")
                        nc.sync.dma_start(
                            gb[:], zred[:].rearrange("(r p) k -> p r k", p=128))
                        gf = rws.tile([128, 2, HR], f32, tag="big2", name="gf")
                        for r in range(2):
                            nc.scalar.activation(gf[:, r, :], gb[:, r, :], AF.Relu)
                        nc.sync.dma_start(
                            g_out.ap().rearrange("(r p) k -> p r k", p=128), gf[:])

    nc.compile()
    return nc


def _get_runner():
    global _RUNNER
    if _RUNNER is None:
        import os, sys
        sys.path.insert(0, "/opt/trn_rl_repo")
        sys.path.insert(0, os.path.dirname(os.path.abspath(__file__)))
        nc = _build()
        Runner = _make_runner_cls()
        _RUNNER = Runner(nc, N_CORES)
    return _RUNNER


def _make_runner_cls():
    """Inline runner (kernel.py must be self-contained)."""
    import jax
    from jax.sharding import Mesh, PartitionSpec
    from jax.experimental.shard_map import shard_map
    from concourse import mybir
    from concourse.bass2jax import (_bass_exec_p, install_neuronx_cc_hook,
                                    partition_id_tensor)

    class Runner:
        def __init__(self, nc, n_cores):
            install_neuronx_cc_hook()
            self.nc = nc
            self.n_cores = n_cores
            pname = nc.partition_id_tensor.name if nc.partition_id_tensor else None
            in_names, out_names, out_avals = [], [], []
            for alloc in nc.m.functions[0].allocations:
                if not isinstance(alloc, mybir.MemoryLocationSet):
                    continue
                name = alloc.memorylocations[0].name
                if alloc.kind == "ExternalInput":
                    if name != pname:
                        in_names.append(name)
                elif alloc.kind == "ExternalOutput":
                    out_names.append(name)
                    out_avals.append(jax.core.ShapedArray(
                        tuple(alloc.tensor_shape), mybir.dt.np(alloc.dtype)))
            self.in_names, self.out_names, self.out_avals = in_names, out_names, out_avals
            all_in = list(in_names) + list(out_names)
            if pname is not None:
                all_in.append(pname)

            def _body(*args):
                operands = list(args)
                if pname is not None:
                    operands.append(partition_id_tensor())
                return tuple(_bass_exec_p.bind(
                    *operands, out_avals=tuple(out_avals),
                    in_names=tuple(all_in), out_names=tuple(out_names),
                    lowering_input_output_aliases=(),
                    sim_require_finite=True, sim_require_nnan=True, nc=nc))

            devices = jax.devices()[:n_cores]
            self.mesh = Mesh(np.asarray(devices), ("core",))
            n_args = len(in_names) + len(out_names)
            self.fn = jax.jit(shard_map(
                _body, mesh=self.mesh,
                in_specs=(PartitionSpec("core"),) * n_args,
                out_specs=(PartitionSpec("core"),) * len(out_names),
                check_rep=False))

        def stage(self, in_maps):
            import jax
            per_core = [[np.asarray(m[n]) for n in self.in_names] for m in in_maps]
            concat = [np.concatenate([per_core[c][i] for c in range(self.n_cores)],
                                     axis=0) for i in range(len(self.in_names))]
            zeros = [np.zeros((self.n_cores * a.shape[0], *a.shape[1:]), a.dtype)
                     for a in self.out_avals]
            return [jax.device_put(x) for x in concat + zeros]

        def run_staged(self, staged):
            import jax
            outs = self.fn(*staged)
            jax.block_until_ready(outs)
            return outs

        def run(self, in_maps):
            outs = self.run_staged(self.stage(in_maps))
            res = []
            for c in range(self.n_cores):
                res.append({n: np.asarray(outs[i]).reshape(
                    self.n_cores, *self.out_avals[i].shape)[c]
                    for i, n in enumerate(self.out_names)})
            return res

    return Runner


def make_in_maps(lr_x, Wq, bq, Wk, bk, Wv, bv, Wskip, bskip,
                 gn_weight, gn_bias, gn_mean_scale):
    x = np.asarray(lr_x, np.float32)
    col = np.zeros((128, 3, 16), np.float32)
    for k, vec in enumerate((np.asarray(bq), np.asarray(bk),
                             np.asarray(bv) + np.asarray(bskip))):
        col[:, k, :] = np.asarray(vec, np.float32).reshape(16, 128).T
    rows = np.ascontiguousarray(np.stack(
        [np.asarray(gn_weight, np.float32), np.asarray(gn_bias, np.float32),
         np.asarray(gn_mean_scale, np.float32)], axis=0))
    base = {
        "x": x,
        "wq": np.asarray(Wq, np.float32),
        "wk": np.asarray(Wk, np.float32),
        "wv": np.asarray(Wv, np.float32),
        "ws": np.asarray(Wskip, np.float32),
        "cols": col,
        "rows": rows,
    }
    in_maps = []
    for c in range(N_CORES):
        m = dict(base)
        m["xo"] = np.ascontiguousarray(x[:, c * NO:(c + 1) * NO])
        in_maps.append(m)
    return in_maps


def kernel(**inputs):
    runner = _get_runner()
    in_maps = make_in_maps(**inputs)
    res = runner.run(in_maps)
    return np.concatenate([res[c]["g"] for c in range(N_CORES)], axis=0)
